# revision 3
# baseline (speedup 1.0000x reference)
"""BiGaBP unfolding (4-layer) Trainium2 Bass kernel.

Pure data parallel per the sharding hint: B=64 samples -> 8 per NeuronCore.
Per core everything is SBUF-resident:

  partition p = (b_local, n)  : 8*16 = 128 partitions
  free      f = (m, k)        : 16*256 = 4096 fp32 per plane

State planes [128,16,256]: Hr,Hi,Xr,Xi,vX,vH.  Per layer:
  mega1 (chunked over k, KC=64): factor-node update (err, xi via m-axis
    reduces), VN_H numerators te2/vt2 (stored), VN_X leave-one-out over n
    via PE block-diag-ones matmul (partition-group sum + broadcast), soft
    demod (ACT tanh) and X/vX update.
  mega2 (chunked over m): VN_H leave-one-out over k via free-dim reduce,
    H/vH update.
Last layer: hard demod (ACT sign) -> Xp/vXp; full-sum VN_H -> Hp/vHp.

Reciprocals use the custom-DVE fast-approx op (~51 ULP), except the final
layer's divides which use the ~2 ULP accurate variant (sign(est) feeds the
hard demod).
"""
import sys
import numpy as np

sys.path.insert(0, "/opt/trn_rl_repo")

import concourse.bass as bass  # noqa: E402
import concourse.tile as tile  # noqa: E402
from concourse import bacc, mybir  # noqa: E402
from concourse.bass_utils import run_bass_kernel_spmd  # noqa: E402

FP = mybir.dt.float32
AX = mybir.AxisListType
OP = mybir.AluOpType
AF = mybir.ActivationFunctionType
MS = bass.MemorySpace

BS, C, N, M, K = 4, 16, 16, 16, 256
CP, KP = 2, 32
B = BS * C
NCORES = 8
BPC = B // NCORES          # 8 samples per core
P = BPC * N                # 128 partitions
S = float(np.float32(0.7071067811865476))
NLAYER = 4
KC = 64                    # k-chunk width in mega1
NCH = K // KC
MC = 4                     # m-chunk width in mega2
S2 = float(np.float32(S) * np.float32(S))

_CACHE = {}


def _bc_k(ap_small, m, kc):
    # [128, kc] -> [128, m, kc] broadcast along m (stride-0 middle axis)
    return ap_small.unsqueeze(1).broadcast_to([P, m, kc])


def _bc_inner(ap_small, kk):
    # [128, m] -> [128, m, kk] broadcast along innermost axis
    return ap_small.unsqueeze(2).broadcast_to([P, ap_small.shape[1], kk])


def _flat(ap3):
    return ap3.rearrange("p m k -> p (m k)")


def _build():
    if "nc" in _CACHE:
        return _CACHE["nc"]
    nc = bacc.Bacc("TRN2", target_bir_lowering=False, debug=False,
                   num_devices=NCORES)

    def din(name, shape):
        return nc.dram_tensor(name, shape, FP, kind="ExternalInput").ap()

    def dout(name, shape):
        return nc.dram_tensor(name, shape, FP, kind="ExternalOutput").ap()

    xr_d = din("xr", [P, M, K])
    xi_d = din("xi", [P, M, K])
    vx_d = din("vx", [P, M, K])
    hr_d = din("hr", [P, M])
    hi_d = din("hi", [P, M])
    vh_d = din("vh", [P, M])
    cdata_d = din("cdata", [P, 5, K])      # yr, yi, n0, mask, smask
    lc_d = din("lc", [P, 10, K])           # per-layer: m2e, om, s2em; + m2e3
    sc_d = din("sc", [P, 16])              # per-layer scalars
    bones_d = din("bones", [P, P])         # block-diag ones

    hp_r_d = dout("hp_r", [P, M])
    hp_i_d = dout("hp_i", [P, M])
    vhp_d = dout("vhp", [P, M])
    xp_r_d = dout("xp_r", [BPC, M, K])
    xp_i_d = dout("xp_i", [BPC, M, K])
    vxp_d = dout("vxp", [BPC, M, K])

    with tile.TileContext(nc) as tc:
        from contextlib import ExitStack
        with ExitStack() as ctx:
            state = ctx.enter_context(tc.tile_pool(name="state", bufs=1))
            cpool = ctx.enter_context(tc.tile_pool(name="cons", bufs=1))
            scr = ctx.enter_context(tc.tile_pool(name="scr", bufs=1))
            sm = ctx.enter_context(tc.tile_pool(name="sm", bufs=1))
            ps = ctx.enter_context(tc.tile_pool(name="ps", bufs=1, space=MS.PSUM))

            V = nc.vector
            A = nc.scalar
            T = nc.tensor
            STT = V.scalar_tensor_tensor

            # ---- persistent tiles ----
            Hr = state.tile([P, M, K], FP, tag="Hr", name="Hr")
            Hi = state.tile([P, M, K], FP, tag="Hi", name="Hi")
            Xr = state.tile([P, M, K], FP, tag="Xr", name="Xr")
            Xi = state.tile([P, M, K], FP, tag="Xi", name="Xi")
            vX = state.tile([P, M, K], FP, tag="vX", name="vX")
            vH = state.tile([P, M, K], FP, tag="vH", name="vH")
            te2r = state.tile([P, M, K], FP, tag="te2r", name="te2r")
            te2i = state.tile([P, M, K], FP, tag="te2i", name="te2i")
            vt2 = state.tile([P, M, K], FP, tag="vt2", name="vt2")

            cdata = cpool.tile([P, 5, K], FP, tag="cdata", name="cdata")
            lc = cpool.tile([P, 10, K], FP, tag="lc", name="lc")
            sc = cpool.tile([P, 16], FP, tag="sc", name="sc")
            bones = cpool.tile([P, P], FP, tag="bones", name="bones")
            hcmp = cpool.tile([P, 3, M], FP, tag="hcmp", name="hcmp")

            # ---- input DMAs ----
            nc.sync.dma_start(cdata[:], cdata_d)
            nc.sync.dma_start(lc[:], lc_d)
            nc.sync.dma_start(sc[:], sc_d)
            nc.sync.dma_start(bones[:], bones_d)
            nc.sync.dma_start(hcmp[:, 0, :], hr_d)
            nc.sync.dma_start(hcmp[:, 1, :], hi_d)
            nc.sync.dma_start(hcmp[:, 2, :], vh_d)
            for cc in range(NCH):
                SL = slice(cc * KC, (cc + 1) * KC)
                nc.sync.dma_start(Xr[:, :, SL], xr_d[:, :, SL])
                nc.sync.dma_start(Xi[:, :, SL], xi_d[:, :, SL])
                nc.sync.dma_start(vX[:, :, SL], vx_d[:, :, SL])

            # broadcast H compact over k (strided-input copies)
            V.tensor_copy(Hr[:], _bc_inner(hcmp[:, 0, :], K))
            A.copy(Hi[:], _bc_inner(hcmp[:, 1, :], K))
            A.copy(vH[:], _bc_inner(hcmp[:, 2, :], K))

            yr = cdata[:, 0, :]
            yi = cdata[:, 1, :]
            n0 = cdata[:, 2, :]
            maskc = cdata[:, 3, :]
            smaskc = cdata[:, 4, :]

            for lay in range(NLAYER):
                last = lay == NLAYER - 1
                if not last:
                    m2e = lc[:, 3 * lay + 0, :]
                    omc = lc[:, 3 * lay + 1, :]
                    s2em = lc[:, 3 * lay + 2, :]
                    cgam = sc[:, lay:lay + 1]
                    eta_s = sc[:, 3 + lay:4 + lay]
                    ometa_s = sc[:, 6 + lay:7 + lay]
                else:
                    m2e = lc[:, 9, :]

                # ================= mega1: per k-chunk =================
                for cc in range(NCH):
                    SL = slice(cc * KC, (cc + 1) * KC)
                    sh = [P, M, KC]
                    Hr_c, Hi_c = Hr[:, :, SL], Hi[:, :, SL]
                    Xr_c, Xi_c = Xr[:, :, SL], Xi[:, :, SL]
                    vX_c, vH_c = vX[:, :, SL], vH[:, :, SL]

                    def st(tag):
                        return scr.tile(sh, FP, tag=tag, name=tag)

                    s1, s2, s3 = st("s1"), st("s2"), st("s3")
                    s4, s5, s6 = st("s4"), st("s5"), st("s6")
                    s7, s8, s9 = st("s7"), st("s8"), st("s9")
                    srm = sm.tile([P, KC], FP, tag="srm", name="srm")
                    sim_ = sm.tile([P, KC], FP, tag="sim", name="sim")
                    stm = sm.tile([P, KC], FP, tag="stm", name="stm")

                    # --- A: HX, err ---
                    V.tensor_mul(s1[:], Hr_c, Xr_c)
                    V.tensor_mul(s2[:], Hi_c, Xi_c)
                    V.tensor_sub(s1[:], s1[:], s2[:])          # HXr
                    V.tensor_mul(s2[:], Hr_c, Xi_c)
                    V.tensor_mul(s3[:], Hi_c, Xr_c)
                    V.tensor_add(s2[:], s2[:], s3[:])          # HXi
                    V.tensor_reduce(srm[:], s1[:].transpose([0, 2, 1]),
                                    axis=AX.X, op=OP.add)
                    V.tensor_reduce(sim_[:], s2[:].transpose([0, 2, 1]),
                                    axis=AX.X, op=OP.add)
                    STT(srm[:], srm[:], -1.0, yr[:, SL], op0=OP.mult, op1=OP.add)
                    STT(sim_[:], sim_[:], -1.0, yi[:, SL], op0=OP.mult, op1=OP.add)
                    V.tensor_add(s1[:], s1[:], _bc_k(srm[:], M, KC))  # err_r
                    V.tensor_add(s2[:], s2[:], _bc_k(sim_[:], M, KC))  # err_i
                    # --- A: magH, XX2, xi ---
                    A.square(s3[:], Hr_c)
                    A.square(s4[:], Hi_c)
                    V.tensor_add(s3[:], s3[:], s4[:])          # magh
                    A.square(s4[:], Xr_c)
                    A.square(s5[:], Xi_c)
                    V.tensor_add(s4[:], s4[:], s5[:])          # xx2
                    V.tensor_add(s5[:], s4[:], vX_c)           # XXv
                    V.tensor_mul(s6[:], s3[:], vX_c)
                    V.tensor_mul(s5[:], vH_c, s5[:])
                    V.tensor_add(s5[:], s6[:], s5[:])          # tmp
                    V.tensor_reduce(stm[:], s5[:].transpose([0, 2, 1]),
                                    axis=AX.X, op=OP.add)
                    V.tensor_add(stm[:], stm[:], n0[:, SL])    # SN
                    STT(s5[:], s5[:], -1.0, _bc_k(stm[:], M, KC),
                        op0=OP.mult, op1=OP.add)               # xi_y
                    V.tensor_add(s6[:], s5[:], vH_c)           # xi_x
                    V.reciprocal_approx_fast(s6[:], s6[:])     # r1
                    V.tensor_add(s5[:], s5[:], vX_c)           # xi_h
                    V.reciprocal_approx_fast(s5[:], s5[:])     # r2
                    STT(s5[:], s5[:], 1.0, _bc_k(m2e[:, SL], M, KC),
                        op0=OP.mult, op1=OP.mult)              # w
                    # --- C-pre: t2, vt2, te2 (old X) ---
                    V.tensor_mul(s7[:], Xr_c, s5[:])           # t2r
                    V.tensor_mul(s8[:], Xi_c, s5[:])           # t2i
                    V.tensor_mul(vt2[:, :, SL], s4[:], s5[:])  # vt2 = xx2*w
                    V.tensor_mul(s4[:], s7[:], s1[:])
                    V.tensor_mul(s9[:], s8[:], s2[:])
                    V.tensor_add(te2r[:, :, SL], s4[:], s9[:])
                    V.tensor_mul(s4[:], s7[:], s2[:])
                    V.tensor_mul(s9[:], s8[:], s1[:])
                    V.tensor_sub(te2i[:, :, SL], s4[:], s9[:])
                    # --- B: t, vt, te, PE block sums ---
                    V.tensor_mul(s5[:], Hr_c, s6[:])           # tr
                    V.tensor_mul(s7[:], Hi_c, s6[:])           # ti
                    V.tensor_mul(s3[:], s3[:], s6[:])          # vt (in magh)
                    psv = ps.tile(sh, FP, tag="psv")
                    psr = ps.tile(sh, FP, tag="psr")
                    psi = ps.tile(sh, FP, tag="psi")
                    s3f, psvf = _flat(s3[:]), _flat(psv[:])
                    for hh in range(2):
                        HS = slice(hh * 512, (hh + 1) * 512)
                        T.matmul(psvf[:, HS], bones[:], s3f[:, HS],
                                 start=True, stop=True)
                    V.tensor_mul(s4[:], s5[:], s1[:])
                    V.tensor_mul(s9[:], s7[:], s2[:])
                    V.tensor_add(s4[:], s4[:], s9[:])          # ter
                    V.tensor_mul(s8[:], s5[:], s2[:])
                    V.tensor_mul(s9[:], s7[:], s1[:])
                    V.tensor_sub(s8[:], s8[:], s9[:])          # tei
                    s4f, s8f = _flat(s4[:]), _flat(s8[:])
                    psrf, psif = _flat(psr[:]), _flat(psi[:])
                    for hh in range(2):
                        HS = slice(hh * 512, (hh + 1) * 512)
                        T.matmul(psrf[:, HS], bones[:], s4f[:, HS],
                                 start=True, stop=True)
                        T.matmul(psif[:, HS], bones[:], s8f[:, HS],
                                 start=True, stop=True)
                    if not last:
                        STT(s3[:], s3[:], -1.0, psv[:], op0=OP.mult, op1=OP.add)
                        V.reciprocal_approx_fast(s3[:], s3[:])  # varX
                        STT(s4[:], s4[:], -1.0, psr[:], op0=OP.mult, op1=OP.add)
                        STT(s8[:], s8[:], -1.0, psi[:], op0=OP.mult, op1=OP.add)
                        V.tensor_mul(s4[:], s4[:], s3[:])       # est_r
                        V.tensor_mul(s8[:], s8[:], s3[:])       # est_i
                        A.activation(s5[:], s4[:], AF.Tanh, scale=cgam)  # mr
                        A.activation(s7[:], s8[:], AF.Tanh, scale=cgam)  # mi
                        omb = _bc_k(omc[:, SL], M, KC)
                        s2eb = _bc_k(s2em[:, SL], M, KC)
                        V.tensor_mul(s1[:], Xr_c, omb)
                        STT(s2[:], s2eb, 2.0 * S, s5[:],
                            op0=OP.mult, op1=OP.mult)           # Sem*mr
                        V.tensor_add(Xr_c, s1[:], s2[:])
                        V.tensor_mul(s1[:], Xi_c, omb)
                        STT(s2[:], s2eb, 2.0 * S, s7[:],
                            op0=OP.mult, op1=OP.mult)
                        V.tensor_add(Xi_c, s1[:], s2[:])
                        A.square(s1[:], s5[:])
                        A.square(s2[:], s7[:])
                        V.tensor_add(s1[:], s1[:], s2[:])       # q2
                        V.tensor_mul(s2[:], vX_c, omb)
                        STT(s3[:], s1[:], -1.0, s2eb,
                            op0=OP.mult, op1=OP.mult)           # -q2*S2em
                        V.tensor_add(s2[:], s2[:], s3[:])
                        STT(vX_c, s2eb, 2.0, s2[:],
                            op0=OP.mult, op1=OP.add)            # + em
                    else:
                        V.reciprocal_approx_accurate(s3[:], psv[:], s9[:])
                        V.tensor_mul(s4[:], psr[:], s3[:])      # est_r
                        V.tensor_mul(s8[:], psi[:], s3[:])      # est_i
                        A.activation(s5[:], s4[:], AF.Sign)     # sgn_r
                        A.activation(s7[:], s8[:], AF.Sign)     # sgn_i
                        smb = _bc_k(smaskc[:, SL], M, KC)
                        V.tensor_mul(s4[:], s5[:], smb)         # Xp_r
                        V.tensor_mul(s8[:], s7[:], smb)         # Xp_i
                        nc.sync.dma_start(xp_r_d[:, :, SL], s4[0:P:N, :, :])
                        nc.sync.dma_start(xp_i_d[:, :, SL], s8[0:P:N, :, :])
                        # vp = (1 - epr^2) - epi^2 (ref op order)
                        V.tensor_scalar_mul(s5[:], s5[:], S)    # epr
                        V.tensor_mul(s5[:], s5[:], s5[:])
                        V.tensor_scalar(s5[:], s5[:], -1.0, 1.0,
                                        op0=OP.mult, op1=OP.add)
                        V.tensor_scalar_mul(s7[:], s7[:], S)
                        V.tensor_mul(s7[:], s7[:], s7[:])
                        V.tensor_sub(s5[:], s5[:], s7[:])       # vp
                        V.tensor_mul(s5[:], s5[:], _bc_k(maskc[:, SL], M, KC))
                        nc.sync.dma_start(vxp_d[:, :, SL], s5[0:P:N, :, :])

                # ================= mega2: VN_H =================
                if not last:
                    for jj in range(M // MC):
                        JS = slice(jj * MC, (jj + 1) * MC)
                        shj = [P, MC, K]
                        v_j = vt2[:, JS, :]
                        r_j = te2r[:, JS, :]
                        i_j = te2i[:, JS, :]
                        sv = sm.tile([P, MC], FP, tag="sv", name="sv")
                        st_ = sm.tile([P, MC], FP, tag="st_", name="st_")
                        si2 = sm.tile([P, MC], FP, tag="si2", name="si2")
                        qg = scr.tile(shj, FP, tag="s1")
                        qh = scr.tile(shj, FP, tag="s2")
                        V.tensor_reduce(sv[:], v_j, axis=AX.X, op=OP.add)
                        V.tensor_reduce(st_[:], r_j, axis=AX.X, op=OP.add)
                        V.tensor_reduce(si2[:], i_j, axis=AX.X, op=OP.add)
                        V.tensor_scalar_add(sv[:], sv[:], 1.0)  # Sv+1
                        STT(qg[:], v_j, -1.0, _bc_inner(sv[:], K),
                            op0=OP.mult, op1=OP.add)
                        V.reciprocal_approx_fast(qg[:], qg[:])
                        V.tensor_scalar(qg[:], qg[:], eta_s, None,
                                        op0=OP.mult)            # gve
                        STT(qh[:], r_j, -1.0, _bc_inner(st_[:], K),
                            op0=OP.mult, op1=OP.add)
                        V.tensor_mul(qh[:], qh[:], qg[:])
                        STT(Hr[:, JS, :], Hr[:, JS, :], ometa_s, qh[:],
                            op0=OP.mult, op1=OP.add)
                        STT(qh[:], i_j, -1.0, _bc_inner(si2[:], K),
                            op0=OP.mult, op1=OP.add)
                        V.tensor_mul(qh[:], qh[:], qg[:])
                        STT(Hi[:, JS, :], Hi[:, JS, :], ometa_s, qh[:],
                            op0=OP.mult, op1=OP.add)
                        STT(vH[:, JS, :], vH[:, JS, :], ometa_s, qg[:],
                            op0=OP.mult, op1=OP.add)
                else:
                    sv = sm.tile([P, M], FP, tag="svL", name="svL")
                    st_ = sm.tile([P, M], FP, tag="stL", name="stL")
                    si2 = sm.tile([P, M], FP, tag="siL", name="siL")
                    g1 = sm.tile([P, M], FP, tag="g1", name="g1")
                    g2 = sm.tile([P, M], FP, tag="g2", name="g2")
                    h1 = sm.tile([P, M], FP, tag="h1", name="h1")
                    V.tensor_reduce(sv[:], vt2[:], axis=AX.X, op=OP.add)
                    V.tensor_reduce(st_[:], te2r[:], axis=AX.X, op=OP.add)
                    V.tensor_reduce(si2[:], te2i[:], axis=AX.X, op=OP.add)
                    V.reciprocal_approx_accurate(g1[:], sv[:], g2[:])  # varH
                    V.tensor_scalar_add(g2[:], g1[:], 1.0)      # 1+varH
                    V.reciprocal_approx_accurate(h1[:], g2[:], sv[:])  # g
                    V.tensor_mul(st_[:], st_[:], g1[:])         # estHr
                    V.tensor_mul(si2[:], si2[:], g1[:])         # estHi
                    V.tensor_mul(st_[:], st_[:], h1[:])         # Hp_r
                    V.tensor_mul(si2[:], si2[:], h1[:])         # Hp_i
                    V.tensor_mul(g1[:], g1[:], h1[:])           # vHp
                    nc.sync.dma_start(hp_r_d, st_[:])
                    nc.sync.dma_start(hp_i_d, si2[:])
                    nc.sync.dma_start(vhp_d, g1[:])

    nc.compile()
    _CACHE["nc"] = nc
    return nc


def _host_prep(inputs):
    H_est = np.asarray(inputs["H_est"])
    X_est = np.asarray(inputs["X_est"])
    var_X = np.asarray(inputs["var_X"], np.float32)
    var_H = np.asarray(inputs["var_H"], np.float32)
    Y = np.asarray(inputs["Y"])
    N0 = np.asarray(inputs["N0"], np.float32)
    alphas = np.asarray(inputs["alphas"], np.float32)
    betas = np.asarray(inputs["betas"], np.float32)
    gammas = np.asarray(inputs["gammas"], np.float32)
    etas = np.asarray(inputs["etas"], np.float32)

    mask_g = np.ones((B, K), np.float32)
    for b in range(B):
        if (b % C) < CP:
            mask_g[b, :KP] = 0.0

    bones = np.zeros((P, P), np.float32)
    for g in range(BPC):
        bones[g * N:(g + 1) * N, g * N:(g + 1) * N] = 1.0

    Sf = np.float32(S)
    in_maps = []
    for c in range(NCORES):
        bsl = slice(c * BPC, (c + 1) * BPC)
        xr = np.broadcast_to(X_est[bsl].real[:, None], (BPC, N, M, K))
        xi = np.broadcast_to(X_est[bsl].imag[:, None], (BPC, N, M, K))
        vx = np.broadcast_to(var_X[bsl][:, None], (BPC, N, M, K))
        maskp = np.repeat(mask_g[bsl], N, axis=0)          # [128, K]
        cdata = np.stack([
            Y[bsl].real.reshape(P, K).astype(np.float32),
            Y[bsl].imag.reshape(P, K).astype(np.float32),
            N0[bsl].reshape(P, K),
            maskp,
            Sf * maskp,
        ], axis=1)
        lc = np.zeros((P, 10, K), np.float32)
        for lay in range(3):
            em = etas[lay] * maskp
            lc[:, 3 * lay + 0] = alphas[lay] * (1.0 - maskp) + betas[lay] * maskp
            lc[:, 3 * lay + 1] = 1.0 - em
            lc[:, 3 * lay + 2] = (Sf * Sf) * em
        lc[:, 9] = alphas[3] * (1.0 - maskp) + betas[3] * maskp
        scm = np.zeros((P, 16), np.float32)
        for lay in range(3):
            scm[:, lay] = np.float32(2.0) * Sf / gammas[lay]
            scm[:, 3 + lay] = etas[lay]
            scm[:, 6 + lay] = np.float32(1.0) - etas[lay]
        in_maps.append({
            "xr": np.ascontiguousarray(xr.reshape(P, M, K), np.float32),
            "xi": np.ascontiguousarray(xi.reshape(P, M, K), np.float32),
            "vx": np.ascontiguousarray(vx.reshape(P, M, K), np.float32),
            "hr": np.ascontiguousarray(H_est[bsl].real.reshape(P, M), np.float32),
            "hi": np.ascontiguousarray(H_est[bsl].imag.reshape(P, M), np.float32),
            "vh": np.ascontiguousarray(var_H[bsl].reshape(P, M), np.float32),
            "cdata": np.ascontiguousarray(cdata),
            "lc": np.ascontiguousarray(lc),
            "sc": np.ascontiguousarray(scm),
            "bones": bones,
        })
    return in_maps


def kernel(**inputs):
    nc = _build()
    in_maps = _host_prep(inputs)
    res = run_bass_kernel_spmd(nc, in_maps, list(range(NCORES))).results
    hp = np.empty((B, N, M), np.complex64)
    xp = np.empty((B, M, K), np.complex64)
    vxp = np.empty((B, M, K), np.float32)
    vhp = np.empty((B, N, M), np.float32)
    for c in range(NCORES):
        bsl = slice(c * BPC, (c + 1) * BPC)
        r = res[c]
        hp[bsl] = (r["hp_r"] + 1j * r["hp_i"]).reshape(BPC, N, M)
        vhp[bsl] = r["vhp"].reshape(BPC, N, M)
        xp[bsl] = r["xp_r"] + 1j * r["xp_i"]
        vxp[bsl] = r["vxp"]
    return hp, xp, vxp, vhp


# revision 7
# speedup vs baseline: 732.0827x; 732.0827x over previous
"""BiGaBP unfolding (4-layer) Trainium2 Bass kernel.

Pure data parallel per the sharding hint: B=64 samples -> 8 per NeuronCore.
Per core everything is SBUF-resident:

  partition p = (b_local, n)  : 8*16 = 128 partitions
  free      f = (m, k)        : 16*256 = 4096 fp32 per plane

State planes [128,16,256]: Hr,Hi,Xr,Xi,vX,vH.  Per layer:
  mega1 (chunked over k, KC=64): factor-node update (err, xi via m-axis
    reduces), VN_H numerators te2/vt2 (stored, factored te2 = w*(X o err)),
    VN_X leave-one-out over n via PE block-diag-ones matmul (partition-group
    sum + broadcast), soft demod (ACT tanh) and X/vX update.
  mega2 (chunked over m): VN_H leave-one-out over k via free-dim reduce,
    H/vH update.
Last layer: hard demod (ACT sign) -> Xp/vXp; full-sum VN_H -> Hp/vHp.

Work is split across three engines: DVE (1x fp32 tensor-tensor), GPSIMD
(parallel independent product streams), ACT (squares, tanh/sign, affine
tensor-scalar ops).  Pilot-mask structure: only k<32 has pilots, so only the
first k-chunk needs mask tensors; chunks 1..3 use scalar-constant forms.

Reciprocals use the custom-DVE fast-approx op (~51 ULP), except the final
layer's divides which use the ~2 ULP accurate variant (sign(est) feeds the
hard demod).
"""
import sys
import numpy as np

sys.path.insert(0, "/opt/trn_rl_repo")

import concourse.bass as bass  # noqa: E402
import concourse.tile as tile  # noqa: E402
from concourse import bacc, mybir  # noqa: E402
from concourse.bass_utils import run_bass_kernel_spmd  # noqa: E402

FP = mybir.dt.float32
AX = mybir.AxisListType
OP = mybir.AluOpType
AF = mybir.ActivationFunctionType
MS = bass.MemorySpace

BS, C, N, M, K = 4, 16, 16, 16, 256
CP, KP = 2, 32
B = BS * C
NCORES = 8
BPC = B // NCORES          # 8 samples per core
P = BPC * N                # 128 partitions
S = float(np.float32(0.7071067811865476))
NLAYER = 4
KC = 64                    # k-chunk width in mega1
NCH = K // KC
MC = 4                     # m-chunk width in mega2

_CACHE = {}


def _bc_k(ap_small, m, kc):
    # [128, kc] -> [128, m, kc] broadcast along m (stride-0 middle axis)
    return ap_small.unsqueeze(1).broadcast_to([P, m, kc])


def _bc_inner(ap_small, kk):
    # [128, m] -> [128, m, kk] broadcast along innermost axis
    return ap_small.unsqueeze(2).broadcast_to([P, ap_small.shape[1], kk])


def _flat(ap3):
    return ap3.rearrange("p m k -> p (m k)")


def _build(reps=1):
    key = f"nc{reps}"
    if key in _CACHE:
        return _CACHE[key]
    nc = bacc.Bacc("TRN2", target_bir_lowering=False, debug=False,
                   num_devices=NCORES)

    def din(name, shape):
        return nc.dram_tensor(name, shape, FP, kind="ExternalInput").ap()

    def dout(name, shape):
        return nc.dram_tensor(name, shape, FP, kind="ExternalOutput").ap()

    xr_d = din("xr", [P, M, K])
    xi_d = din("xi", [P, M, K])
    vx_d = din("vx", [P, M, K])
    hr_d = din("hr", [P, M])
    hi_d = din("hi", [P, M])
    vh_d = din("vh", [P, M])
    cdata_d = din("cdata", [P, 5, K])      # yr, yi, n0, mask, smask
    lc_d = din("lc", [P, 10, K])           # per-layer: m2e, om, s2em; + m2e3
    sc_d = din("sc", [P, 24])              # per-layer scalars
    bones_d = din("bones", [P, P])         # block-diag ones

    hp_r_d = dout("hp_r", [P, M])
    hp_i_d = dout("hp_i", [P, M])
    vhp_d = dout("vhp", [P, M])
    xp_r_d = dout("xp_r", [BPC, M, K])
    xp_i_d = dout("xp_i", [BPC, M, K])
    vxp_d = dout("vxp", [BPC, M, K])

    with tile.TileContext(nc) as tc:
        from contextlib import ExitStack
        with ExitStack() as ctx:
            state = ctx.enter_context(tc.tile_pool(name="state", bufs=1))
            cpool = ctx.enter_context(tc.tile_pool(name="cons", bufs=1))
            scr = ctx.enter_context(tc.tile_pool(name="scr", bufs=1))
            sm = ctx.enter_context(tc.tile_pool(name="sm", bufs=1))
            ps = ctx.enter_context(tc.tile_pool(name="ps", bufs=1, space=MS.PSUM))

            V = nc.vector
            A = nc.scalar
            G = nc.gpsimd
            T = nc.tensor
            STT = V.scalar_tensor_tensor

            for _rep in range(reps):
                _emit(nc, tc, state, cpool, scr, sm, ps, V, A, G, T, STT,
                      xr_d, xi_d, vx_d, hr_d, hi_d, vh_d, cdata_d, lc_d,
                      sc_d, bones_d, hp_r_d, hp_i_d, vhp_d, xp_r_d, xp_i_d,
                      vxp_d)

    nc.compile()
    _CACHE[key] = nc
    return nc


def _emit(nc, tc, state, cpool, scr, sm, ps, V, A, G, T, STT,
          xr_d, xi_d, vx_d, hr_d, hi_d, vh_d, cdata_d, lc_d, sc_d, bones_d,
          hp_r_d, hp_i_d, vhp_d, xp_r_d, xp_i_d, vxp_d):
    # ---- persistent tiles ----
    Hr = state.tile([P, M, K], FP, tag="Hr", name="Hr")
    Hi = state.tile([P, M, K], FP, tag="Hi", name="Hi")
    Xr = state.tile([P, M, K], FP, tag="Xr", name="Xr")
    Xi = state.tile([P, M, K], FP, tag="Xi", name="Xi")
    vX = state.tile([P, M, K], FP, tag="vX", name="vX")
    vH = state.tile([P, M, K], FP, tag="vH", name="vH")
    te2r = state.tile([P, M, K], FP, tag="te2r", name="te2r")
    te2i = state.tile([P, M, K], FP, tag="te2i", name="te2i")
    vt2 = state.tile([P, M, K], FP, tag="vt2", name="vt2")

    cdata = cpool.tile([P, 5, K], FP, tag="cdata", name="cdata")
    lc = cpool.tile([P, 10, K], FP, tag="lc", name="lc")
    sc = cpool.tile([P, 24], FP, tag="sc", name="sc")
    bones = cpool.tile([P, P], FP, tag="bones", name="bones")
    hcmp = cpool.tile([P, 3, M], FP, tag="hcmp", name="hcmp")

    # ---- input DMAs ----
    nc.sync.dma_start(cdata[:], cdata_d)
    nc.sync.dma_start(lc[:], lc_d)
    nc.sync.dma_start(sc[:], sc_d)
    nc.sync.dma_start(bones[:], bones_d)
    nc.sync.dma_start(hcmp[:, 0, :], hr_d)
    nc.sync.dma_start(hcmp[:, 1, :], hi_d)
    nc.sync.dma_start(hcmp[:, 2, :], vh_d)
    for cc in range(NCH):
        SL = slice(cc * KC, (cc + 1) * KC)
        nc.sync.dma_start(Xr[:, :, SL], xr_d[:, :, SL])
        nc.sync.dma_start(Xi[:, :, SL], xi_d[:, :, SL])
        nc.sync.dma_start(vX[:, :, SL], vx_d[:, :, SL])

    # broadcast H compact over k (strided-input copies)
    G.tensor_copy(Hr[:], _bc_inner(hcmp[:, 0, :], K))
    A.copy(Hi[:], _bc_inner(hcmp[:, 1, :], K))
    A.copy(vH[:], _bc_inner(hcmp[:, 2, :], K))

    yr = cdata[:, 0, :]
    yi = cdata[:, 1, :]
    n0 = cdata[:, 2, :]
    maskc = cdata[:, 3, :]
    smaskc = cdata[:, 4, :]

    for lay in range(NLAYER):
        last = lay == NLAYER - 1
        if not last:
            m2e = lc[:, 3 * lay + 0, :]
            omc = lc[:, 3 * lay + 1, :]
            s2em = lc[:, 3 * lay + 2, :]
            cgam = sc[:, lay:lay + 1]
            eta_s = sc[:, 3 + lay:4 + lay]
            ometa_s = sc[:, 6 + lay:7 + lay]
            seta_s = sc[:, 9 + lay:10 + lay]
            ns2e_s = sc[:, 12 + lay:13 + lay]
            beta_s = sc[:, 15 + lay:16 + lay]
        else:
            m2e = lc[:, 9, :]
            beta_s = sc[:, 18:19]

        # ================= mega1: per k-chunk =================
        for cc in range(NCH):
            chunk0 = cc == 0
            SL = slice(cc * KC, (cc + 1) * KC)
            sh = [P, M, KC]
            Hr_c, Hi_c = Hr[:, :, SL], Hi[:, :, SL]
            Xr_c, Xi_c = Xr[:, :, SL], Xi[:, :, SL]
            vX_c, vH_c = vX[:, :, SL], vH[:, :, SL]

            def st(tag):
                return scr.tile(sh, FP, tag=tag, name=tag)

            s1, s2, s3 = st("s1"), st("s2"), st("s3")
            s4, s5, s6 = st("s4"), st("s5"), st("s6")
            s7, s8, s9 = st("s7"), st("s8"), st("s9")
            srm = sm.tile([P, KC], FP, tag="srm", name="srm")
            sim_ = sm.tile([P, KC], FP, tag="sim", name="sim")
            stm = sm.tile([P, KC], FP, tag="stm", name="stm")

            # --- A: HX, err ---
            V.tensor_mul(s1[:], Hr_c, Xr_c)
            G.tensor_mul(s2[:], Hi_c, Xi_c)
            V.tensor_sub(s1[:], s1[:], s2[:])          # HXr
            V.tensor_mul(s2[:], Hr_c, Xi_c)
            G.tensor_mul(s3[:], Hi_c, Xr_c)
            V.tensor_add(s2[:], s2[:], s3[:])          # HXi
            V.tensor_reduce(srm[:], s1[:].transpose([0, 2, 1]),
                            axis=AX.X, op=OP.add)
            V.tensor_reduce(sim_[:], s2[:].transpose([0, 2, 1]),
                            axis=AX.X, op=OP.add)
            STT(srm[:], srm[:], -1.0, yr[:, SL], op0=OP.mult, op1=OP.add)
            STT(sim_[:], sim_[:], -1.0, yi[:, SL], op0=OP.mult, op1=OP.add)
            G.tensor_add(s1[:], s1[:], _bc_k(srm[:], M, KC))  # err_r
            G.tensor_add(s2[:], s2[:], _bc_k(sim_[:], M, KC))  # err_i
            # --- A: magH, XX2, xi ---
            A.square(s3[:], Hr_c)
            A.square(s4[:], Hi_c)
            V.tensor_add(s3[:], s3[:], s4[:])          # magh
            A.square(s4[:], Xr_c)
            A.square(s5[:], Xi_c)
            V.tensor_add(s4[:], s4[:], s5[:])          # xx2
            V.tensor_add(s5[:], s4[:], vX_c)           # XXv
            G.tensor_mul(s6[:], s3[:], vX_c)           # magh*vX
            V.tensor_mul(s5[:], vH_c, s5[:])
            V.tensor_add(s5[:], s6[:], s5[:])          # tmp
            V.tensor_reduce(stm[:], s5[:].transpose([0, 2, 1]),
                            axis=AX.X, op=OP.add)
            V.tensor_add(stm[:], stm[:], n0[:, SL])    # SN
            STT(s5[:], s5[:], -1.0, _bc_k(stm[:], M, KC),
                op0=OP.mult, op1=OP.add)               # xi_y
            V.tensor_add(s6[:], s5[:], vH_c)           # xi_x
            V.reciprocal_approx_fast(s6[:], s6[:])     # r1
            V.tensor_add(s5[:], s5[:], vX_c)           # xi_h
            V.reciprocal_approx_fast(s5[:], s5[:])     # r2
            if chunk0:
                STT(s5[:], s5[:], 1.0, _bc_k(m2e[:, SL], M, KC),
                    op0=OP.mult, op1=OP.mult)          # w = r2*m2
            # --- C-pre (factored): te2 = w*(X o err), vt2 = w*xx2 ---
            V.tensor_mul(s7[:], Xr_c, s1[:])           # A1
            G.tensor_mul(s8[:], Xi_c, s2[:])           # A2
            V.tensor_add(s7[:], s7[:], s8[:])          # A3
            G.tensor_mul(s8[:], Xr_c, s2[:])           # A4
            G.tensor_mul(s9[:], Xi_c, s1[:])           # A5
            G.tensor_sub(s8[:], s8[:], s9[:])          # A6
            if chunk0:
                V.tensor_mul(te2r[:, :, SL], s7[:], s5[:])
                V.tensor_mul(te2i[:, :, SL], s8[:], s5[:])
                V.tensor_mul(vt2[:, :, SL], s4[:], s5[:])
            else:
                STT(te2r[:, :, SL], s7[:], beta_s, s5[:],
                    op0=OP.mult, op1=OP.mult)
                STT(te2i[:, :, SL], s8[:], beta_s, s5[:],
                    op0=OP.mult, op1=OP.mult)
                STT(vt2[:, :, SL], s4[:], beta_s, s5[:],
                    op0=OP.mult, op1=OP.mult)
            # --- B (factored): te = r1*(H o err), vt = magh*r1 ---
            V.tensor_mul(s4[:], Hr_c, s1[:])           # B1
            G.tensor_mul(s7[:], Hi_c, s2[:])           # B2
            V.tensor_add(s4[:], s4[:], s7[:])          # B3
            G.tensor_mul(s7[:], Hr_c, s2[:])           # B4
            G.tensor_mul(s8[:], Hi_c, s1[:])           # B5
            G.tensor_sub(s7[:], s7[:], s8[:])          # B6
            G.tensor_mul(s3[:], s3[:], s6[:])          # vt = magh*r1
            V.tensor_mul(s4[:], s4[:], s6[:])          # ter
            V.tensor_mul(s7[:], s7[:], s6[:])          # tei
            psv = ps.tile(sh, FP, tag="psv", name="psv")
            psr = ps.tile(sh, FP, tag="psr", name="psr")
            psi = ps.tile(sh, FP, tag="psi", name="psi")
            for src, dst in ((s3, psv), (s4, psr), (s7, psi)):
                sf, df = _flat(src[:]), _flat(dst[:])
                for hh in range(2):
                    HS = slice(hh * 512, (hh + 1) * 512)
                    T.matmul(df[:, HS], bones[:], sf[:, HS],
                             start=True, stop=True)
            if not last:
                STT(s3[:], s3[:], -1.0, psv[:], op0=OP.mult, op1=OP.add)
                V.reciprocal_approx_fast(s3[:], s3[:])  # varX
                STT(s4[:], s4[:], -1.0, psr[:], op0=OP.mult, op1=OP.add)
                STT(s7[:], s7[:], -1.0, psi[:], op0=OP.mult, op1=OP.add)
                V.tensor_mul(s4[:], s4[:], s3[:])       # est_r
                V.tensor_mul(s7[:], s7[:], s3[:])       # est_i
                A.activation(s5[:], s4[:], AF.Tanh, scale=cgam)  # mr
                A.activation(s8[:], s7[:], AF.Tanh, scale=cgam)  # mi
                if chunk0:
                    omb = _bc_k(omc[:, SL], M, KC)
                    s2eb = _bc_k(s2em[:, SL], M, KC)
                    V.tensor_mul(s1[:], Xr_c, omb)
                    STT(s2[:], s2eb, 2.0 * S, s5[:], op0=OP.mult, op1=OP.mult)
                    V.tensor_add(Xr_c, s1[:], s2[:])
                    V.tensor_mul(s1[:], Xi_c, omb)
                    STT(s2[:], s2eb, 2.0 * S, s8[:], op0=OP.mult, op1=OP.mult)
                    V.tensor_add(Xi_c, s1[:], s2[:])
                    A.square(s1[:], s5[:])
                    A.square(s2[:], s8[:])
                    G.tensor_add(s1[:], s1[:], s2[:])   # q2
                    V.tensor_mul(s2[:], vX_c, omb)
                    STT(s6[:], s1[:], -1.0, s2eb, op0=OP.mult, op1=OP.mult)
                    V.tensor_add(s2[:], s2[:], s6[:])
                    STT(vX_c, s2eb, 2.0, s2[:], op0=OP.mult, op1=OP.add)
                else:
                    A.activation(s1[:], s5[:], AF.Identity, scale=seta_s)
                    STT(Xr_c, Xr_c, ometa_s, s1[:], op0=OP.mult, op1=OP.add)
                    A.activation(s1[:], s8[:], AF.Identity, scale=seta_s)
                    STT(Xi_c, Xi_c, ometa_s, s1[:], op0=OP.mult, op1=OP.add)
                    A.square(s1[:], s5[:])
                    A.square(s2[:], s8[:])
                    G.tensor_add(s1[:], s1[:], s2[:])   # q2
                    A.activation(s2[:], s1[:], AF.Identity,
                                 scale=ns2e_s, bias=eta_s)
                    STT(vX_c, vX_c, ometa_s, s2[:], op0=OP.mult, op1=OP.add)
            else:
                V.reciprocal_approx_accurate(s3[:], psv[:], s9[:])
                V.tensor_mul(s4[:], psr[:], s3[:])      # est_r
                V.tensor_mul(s7[:], psi[:], s3[:])      # est_i
                A.activation(s5[:], s4[:], AF.Sign)     # sgn_r
                A.activation(s8[:], s7[:], AF.Sign)     # sgn_i
                smb = _bc_k(smaskc[:, SL], M, KC)
                V.tensor_mul(s4[:], s5[:], smb)         # Xp_r
                V.tensor_mul(s7[:], s8[:], smb)         # Xp_i
                nc.sync.dma_start(xp_r_d[:, :, SL], s4[0:P:N, :, :])
                nc.sync.dma_start(xp_i_d[:, :, SL], s7[0:P:N, :, :])
                # vp = (1 - epr^2) - epi^2 (ref op order)
                A.mul(s5[:], s5[:], S)                  # epr
                V.tensor_mul(s5[:], s5[:], s5[:])
                A.activation(s5[:], s5[:], AF.Identity, scale=-1.0, bias=1.0)
                A.mul(s8[:], s8[:], S)
                G.tensor_mul(s8[:], s8[:], s8[:])
                V.tensor_sub(s5[:], s5[:], s8[:])       # vp
                V.tensor_mul(s5[:], s5[:], _bc_k(maskc[:, SL], M, KC))
                nc.sync.dma_start(vxp_d[:, :, SL], s5[0:P:N, :, :])

        # ================= mega2: VN_H =================
        if not last:
            for jj in range(M // MC):
                JS = slice(jj * MC, (jj + 1) * MC)
                shj = [P, MC, K]
                v_j = vt2[:, JS, :]
                r_j = te2r[:, JS, :]
                i_j = te2i[:, JS, :]
                sv = sm.tile([P, MC], FP, tag="sv", name="sv")
                st_ = sm.tile([P, MC], FP, tag="st_", name="st_")
                si2 = sm.tile([P, MC], FP, tag="si2", name="si2")
                qg = scr.tile(shj, FP, tag="s1", name="qg")
                qh = scr.tile(shj, FP, tag="s2", name="qh")
                qi = scr.tile(shj, FP, tag="s3", name="qi")
                V.tensor_reduce(sv[:], v_j, axis=AX.X, op=OP.add)
                V.tensor_reduce(st_[:], r_j, axis=AX.X, op=OP.add)
                V.tensor_reduce(si2[:], i_j, axis=AX.X, op=OP.add)
                A.add(sv[:], sv[:], 1.0)                # Sv+1
                STT(qg[:], v_j, -1.0, _bc_inner(sv[:], K),
                    op0=OP.mult, op1=OP.add)
                V.reciprocal_approx_fast(qg[:], qg[:])
                A.activation(qg[:], qg[:], AF.Identity, scale=eta_s)  # gve
                STT(qh[:], r_j, -1.0, _bc_inner(st_[:], K),
                    op0=OP.mult, op1=OP.add)
                V.tensor_mul(qh[:], qh[:], qg[:])
                G.scalar_tensor_tensor(qi[:], i_j, -1.0,
                                       _bc_inner(si2[:], K),
                                       op0=OP.mult, op1=OP.add)
                G.tensor_mul(qi[:], qi[:], qg[:])
                G.scalar_tensor_tensor(Hr[:, JS, :], Hr[:, JS, :], ometa_s,
                                       qh[:], op0=OP.mult, op1=OP.add)
                STT(Hi[:, JS, :], Hi[:, JS, :], ometa_s, qi[:],
                    op0=OP.mult, op1=OP.add)
                G.scalar_tensor_tensor(vH[:, JS, :], vH[:, JS, :], ometa_s,
                                       qg[:], op0=OP.mult, op1=OP.add)
        else:
            sv = sm.tile([P, M], FP, tag="svL", name="sv")
            st_ = sm.tile([P, M], FP, tag="stL", name="st_")
            si2 = sm.tile([P, M], FP, tag="siL", name="si2")
            g1 = sm.tile([P, M], FP, tag="g1", name="g1")
            g2 = sm.tile([P, M], FP, tag="g2", name="g2")
            h1 = sm.tile([P, M], FP, tag="h1", name="h1")
            V.tensor_reduce(sv[:], vt2[:], axis=AX.X, op=OP.add)
            V.tensor_reduce(st_[:], te2r[:], axis=AX.X, op=OP.add)
            V.tensor_reduce(si2[:], te2i[:], axis=AX.X, op=OP.add)
            V.reciprocal_approx_accurate(g1[:], sv[:], g2[:])  # varH
            A.add(g2[:], g1[:], 1.0)                    # 1+varH
            V.reciprocal_approx_accurate(h1[:], g2[:], sv[:])  # g
            V.tensor_mul(st_[:], st_[:], g1[:])         # estHr
            V.tensor_mul(si2[:], si2[:], g1[:])         # estHi
            V.tensor_mul(st_[:], st_[:], h1[:])         # Hp_r
            V.tensor_mul(si2[:], si2[:], h1[:])         # Hp_i
            V.tensor_mul(g1[:], g1[:], h1[:])           # vHp
            nc.sync.dma_start(hp_r_d, st_[:])
            nc.sync.dma_start(hp_i_d, si2[:])
            nc.sync.dma_start(vhp_d, g1[:])


def _host_prep(inputs):
    H_est = np.asarray(inputs["H_est"])
    X_est = np.asarray(inputs["X_est"])
    var_X = np.asarray(inputs["var_X"], np.float32)
    var_H = np.asarray(inputs["var_H"], np.float32)
    Y = np.asarray(inputs["Y"])
    N0 = np.asarray(inputs["N0"], np.float32)
    alphas = np.asarray(inputs["alphas"], np.float32)
    betas = np.asarray(inputs["betas"], np.float32)
    gammas = np.asarray(inputs["gammas"], np.float32)
    etas = np.asarray(inputs["etas"], np.float32)

    mask_g = np.ones((B, K), np.float32)
    for b in range(B):
        if (b % C) < CP:
            mask_g[b, :KP] = 0.0

    bones = np.zeros((P, P), np.float32)
    for g in range(BPC):
        bones[g * N:(g + 1) * N, g * N:(g + 1) * N] = 1.0

    Sf = np.float32(S)
    in_maps = []
    for c in range(NCORES):
        bsl = slice(c * BPC, (c + 1) * BPC)
        xr = np.broadcast_to(X_est[bsl].real[:, None], (BPC, N, M, K))
        xi = np.broadcast_to(X_est[bsl].imag[:, None], (BPC, N, M, K))
        vx = np.broadcast_to(var_X[bsl][:, None], (BPC, N, M, K))
        maskp = np.repeat(mask_g[bsl], N, axis=0)          # [128, K]
        cdata = np.stack([
            Y[bsl].real.reshape(P, K).astype(np.float32),
            Y[bsl].imag.reshape(P, K).astype(np.float32),
            N0[bsl].reshape(P, K),
            maskp,
            Sf * maskp,
        ], axis=1)
        lc = np.zeros((P, 10, K), np.float32)
        for lay in range(3):
            em = etas[lay] * maskp
            lc[:, 3 * lay + 0] = alphas[lay] * (1.0 - maskp) + betas[lay] * maskp
            lc[:, 3 * lay + 1] = 1.0 - em
            lc[:, 3 * lay + 2] = (Sf * Sf) * em
        lc[:, 9] = alphas[3] * (1.0 - maskp) + betas[3] * maskp
        scm = np.zeros((P, 24), np.float32)
        for lay in range(3):
            scm[:, lay] = np.float32(2.0) * Sf / gammas[lay]
            scm[:, 3 + lay] = etas[lay]
            scm[:, 6 + lay] = np.float32(1.0) - etas[lay]
            scm[:, 9 + lay] = Sf * etas[lay]
            scm[:, 12 + lay] = -(Sf * Sf) * etas[lay]
            scm[:, 15 + lay] = betas[lay]
        scm[:, 18] = betas[3]
        in_maps.append({
            "xr": np.ascontiguousarray(xr.reshape(P, M, K), np.float32),
            "xi": np.ascontiguousarray(xi.reshape(P, M, K), np.float32),
            "vx": np.ascontiguousarray(vx.reshape(P, M, K), np.float32),
            "hr": np.ascontiguousarray(H_est[bsl].real.reshape(P, M), np.float32),
            "hi": np.ascontiguousarray(H_est[bsl].imag.reshape(P, M), np.float32),
            "vh": np.ascontiguousarray(var_H[bsl].reshape(P, M), np.float32),
            "cdata": np.ascontiguousarray(cdata),
            "lc": np.ascontiguousarray(lc),
            "sc": np.ascontiguousarray(scm),
            "bones": bones,
        })
    return in_maps


def kernel(**inputs):
    nc = _build()
    in_maps = _host_prep(inputs)
    res = run_bass_kernel_spmd(nc, in_maps, list(range(NCORES))).results
    hp = np.empty((B, N, M), np.complex64)
    xp = np.empty((B, M, K), np.complex64)
    vxp = np.empty((B, M, K), np.float32)
    vhp = np.empty((B, N, M), np.float32)
    for c in range(NCORES):
        bsl = slice(c * BPC, (c + 1) * BPC)
        r = res[c]
        hp[bsl] = (r["hp_r"] + 1j * r["hp_i"]).reshape(BPC, N, M)
        vhp[bsl] = r["vhp"].reshape(BPC, N, M)
        xp[bsl] = r["xp_r"] + 1j * r["xp_i"]
        vxp[bsl] = r["vxp"]
    return hp, xp, vxp, vhp


# revision 8
# speedup vs baseline: 14062.6801x; 19.2091x over previous
"""BiGaBP unfolding (4-layer) Trainium2 Bass kernel.

Pure data parallel per the sharding hint: B=64 samples -> 8 per NeuronCore.
Per core everything is SBUF-resident:

  partition p = (b_local, n)  : 8*16 = 128 partitions
  free      f = (m, k)        : 16*256 = 4096 fp32 per plane

State planes [128,16,256]: Hr,Hi,Xr,Xi,vX,vH.  Per layer:
  mega1 (chunked over k, KC=64): factor-node update (err, xi via m-axis
    reduces), VN_H numerators te2/vt2 (stored, factored te2 = w*(X o err)),
    VN_X leave-one-out over n via PE block-diag-ones matmul (partition-group
    sum + broadcast), soft demod (ACT tanh) and X/vX update.
  mega2 (chunked over m): VN_H leave-one-out over k via free-dim reduce,
    H/vH update.
Last layer: hard demod (ACT sign) -> Xp/vXp; full-sum VN_H -> Hp/vHp.

Work is split across three engines: DVE (1x fp32 tensor-tensor), GPSIMD
(parallel independent product streams), ACT (squares, tanh/sign, affine
tensor-scalar ops).  Pilot-mask structure: only k<32 has pilots, so only the
first k-chunk needs mask tensors; chunks 1..3 use scalar-constant forms.

Reciprocals use the custom-DVE fast-approx op (~51 ULP), except the final
layer's divides which use the ~2 ULP accurate variant (sign(est) feeds the
hard demod).
"""
import sys
import numpy as np

sys.path.insert(0, "/opt/trn_rl_repo")

import concourse.bass as bass  # noqa: E402
import concourse.tile as tile  # noqa: E402
from concourse import bacc, mybir  # noqa: E402
from concourse.bass_utils import run_bass_kernel_spmd  # noqa: E402

FP = mybir.dt.float32
AX = mybir.AxisListType
OP = mybir.AluOpType
AF = mybir.ActivationFunctionType
MS = bass.MemorySpace

BS, C, N, M, K = 4, 16, 16, 16, 256
CP, KP = 2, 32
B = BS * C
NCORES = 8
BPC = B // NCORES          # 8 samples per core
P = BPC * N                # 128 partitions
S = float(np.float32(0.7071067811865476))
NLAYER = 4
KC = 64                    # k-chunk width in mega1
NCH = K // KC
MC = 4                     # m-chunk width in mega2

_CACHE = {}


def _bc_k(ap_small, m, kc):
    # [128, kc] -> [128, m, kc] broadcast along m (stride-0 middle axis)
    return ap_small.unsqueeze(1).broadcast_to([P, m, kc])


def _bc_inner(ap_small, kk):
    # [128, m] -> [128, m, kk] broadcast along innermost axis
    return ap_small.unsqueeze(2).broadcast_to([P, ap_small.shape[1], kk])


def _flat(ap3):
    return ap3.rearrange("p m k -> p (m k)")


def _build(reps=1):
    key = f"nc{reps}"
    if key in _CACHE:
        return _CACHE[key]
    nc = bacc.Bacc("TRN2", target_bir_lowering=False, debug=False,
                   num_devices=NCORES)

    def din(name, shape):
        return nc.dram_tensor(name, shape, FP, kind="ExternalInput").ap()

    def dout(name, shape):
        return nc.dram_tensor(name, shape, FP, kind="ExternalOutput").ap()

    xr_d = din("xr", [P, M, K])
    xi_d = din("xi", [P, M, K])
    vx_d = din("vx", [P, M, K])
    hr_d = din("hr", [P, M])
    hi_d = din("hi", [P, M])
    vh_d = din("vh", [P, M])
    cdata_d = din("cdata", [P, 5, K])      # yr, yi, n0, mask, smask
    lc_d = din("lc", [P, 10, K])           # per-layer: m2e, om, s2em; + m2e3
    sc_d = din("sc", [P, 24])              # per-layer scalars
    bones_d = din("bones", [P, P])         # block-diag ones

    hp_r_d = dout("hp_r", [P, M])
    hp_i_d = dout("hp_i", [P, M])
    vhp_d = dout("vhp", [P, M])
    xp_r_d = dout("xp_r", [BPC, M, K])
    xp_i_d = dout("xp_i", [BPC, M, K])
    vxp_d = dout("vxp", [BPC, M, K])

    with tile.TileContext(nc) as tc:
        from contextlib import ExitStack
        with ExitStack() as ctx:
            state = ctx.enter_context(tc.tile_pool(name="state", bufs=1))
            cpool = ctx.enter_context(tc.tile_pool(name="cons", bufs=1))
            scr = ctx.enter_context(tc.tile_pool(name="scr", bufs=1))
            sm = ctx.enter_context(tc.tile_pool(name="sm", bufs=1))
            ps = ctx.enter_context(tc.tile_pool(name="ps", bufs=1, space=MS.PSUM))

            V = nc.vector
            A = nc.scalar
            G = nc.gpsimd
            T = nc.tensor
            STT = V.scalar_tensor_tensor

            for _rep in range(reps):
                _emit(nc, tc, state, cpool, scr, sm, ps, V, A, G, T, STT,
                      xr_d, xi_d, vx_d, hr_d, hi_d, vh_d, cdata_d, lc_d,
                      sc_d, bones_d, hp_r_d, hp_i_d, vhp_d, xp_r_d, xp_i_d,
                      vxp_d)

    nc.compile()
    _CACHE[key] = nc
    return nc


def _emit(nc, tc, state, cpool, scr, sm, ps, V, A, G, T, STT,
          xr_d, xi_d, vx_d, hr_d, hi_d, vh_d, cdata_d, lc_d, sc_d, bones_d,
          hp_r_d, hp_i_d, vhp_d, xp_r_d, xp_i_d, vxp_d):
    # ---- persistent tiles ----
    Hr = state.tile([P, M, K], FP, tag="Hr", name="Hr")
    Hi = state.tile([P, M, K], FP, tag="Hi", name="Hi")
    Xr = state.tile([P, M, K], FP, tag="Xr", name="Xr")
    Xi = state.tile([P, M, K], FP, tag="Xi", name="Xi")
    vX = state.tile([P, M, K], FP, tag="vX", name="vX")
    vH = state.tile([P, M, K], FP, tag="vH", name="vH")
    te2r = state.tile([P, M, K], FP, tag="te2r", name="te2r")
    te2i = state.tile([P, M, K], FP, tag="te2i", name="te2i")
    vt2 = state.tile([P, M, K], FP, tag="vt2", name="vt2")

    cdata = cpool.tile([P, 5, K], FP, tag="cdata", name="cdata")
    lc = cpool.tile([P, 10, K], FP, tag="lc", name="lc")
    sc = cpool.tile([P, 24], FP, tag="sc", name="sc")
    bones = cpool.tile([P, P], FP, tag="bones", name="bones")
    hcmp = cpool.tile([P, 3, M], FP, tag="hcmp", name="hcmp")

    # ---- input DMAs ----
    nc.sync.dma_start(cdata[:], cdata_d)
    nc.sync.dma_start(lc[:], lc_d)
    nc.sync.dma_start(sc[:], sc_d)
    nc.sync.dma_start(bones[:], bones_d)
    nc.sync.dma_start(hcmp[:, 0, :], hr_d)
    nc.sync.dma_start(hcmp[:, 1, :], hi_d)
    nc.sync.dma_start(hcmp[:, 2, :], vh_d)
    for cc in range(NCH):
        SL = slice(cc * KC, (cc + 1) * KC)
        nc.sync.dma_start(Xr[:, :, SL], xr_d[:, :, SL])
        nc.sync.dma_start(Xi[:, :, SL], xi_d[:, :, SL])
        nc.sync.dma_start(vX[:, :, SL], vx_d[:, :, SL])

    # broadcast H compact over k (strided-input copies)
    G.tensor_copy(Hr[:], _bc_inner(hcmp[:, 0, :], K))
    A.copy(Hi[:], _bc_inner(hcmp[:, 1, :], K))
    A.copy(vH[:], _bc_inner(hcmp[:, 2, :], K))

    yr = cdata[:, 0, :]
    yi = cdata[:, 1, :]
    n0 = cdata[:, 2, :]
    maskc = cdata[:, 3, :]
    smaskc = cdata[:, 4, :]

    for lay in range(NLAYER):
        last = lay == NLAYER - 1
        if not last:
            m2e = lc[:, 3 * lay + 0, :]
            omc = lc[:, 3 * lay + 1, :]
            s2em = lc[:, 3 * lay + 2, :]
            cgam = sc[:, lay:lay + 1]
            eta_s = sc[:, 3 + lay:4 + lay]
            ometa_s = sc[:, 6 + lay:7 + lay]
            seta_s = sc[:, 9 + lay:10 + lay]
            ns2e_s = sc[:, 12 + lay:13 + lay]
            beta_s = sc[:, 15 + lay:16 + lay]
        else:
            m2e = lc[:, 9, :]
            beta_s = sc[:, 18:19]

        # ================= mega1: per k-chunk =================
        for cc in range(NCH):
            chunk0 = cc == 0
            SL = slice(cc * KC, (cc + 1) * KC)
            sh = [P, M, KC]
            Hr_c, Hi_c = Hr[:, :, SL], Hi[:, :, SL]
            Xr_c, Xi_c = Xr[:, :, SL], Xi[:, :, SL]
            vX_c, vH_c = vX[:, :, SL], vH[:, :, SL]

            def st(tag):
                return scr.tile(sh, FP, tag=tag, name=tag)

            s1, s2, s3 = st("s1"), st("s2"), st("s3")
            s4, s5, s6 = st("s4"), st("s5"), st("s6")
            s7, s8, s9 = st("s7"), st("s8"), st("s9")
            srm = sm.tile([P, KC], FP, tag="srm", name="srm")
            sim_ = sm.tile([P, KC], FP, tag="sim", name="sim")
            stm = sm.tile([P, KC], FP, tag="stm", name="stm")

            # --- A: HX, err ---
            V.tensor_mul(s1[:], Hr_c, Xr_c)
            G.tensor_mul(s2[:], Hi_c, Xi_c)
            V.tensor_sub(s1[:], s1[:], s2[:])          # HXr
            V.tensor_mul(s2[:], Hr_c, Xi_c)
            G.tensor_mul(s3[:], Hi_c, Xr_c)
            V.tensor_add(s2[:], s2[:], s3[:])          # HXi
            V.tensor_reduce(srm[:], s1[:].transpose([0, 2, 1]),
                            axis=AX.X, op=OP.add)
            V.tensor_reduce(sim_[:], s2[:].transpose([0, 2, 1]),
                            axis=AX.X, op=OP.add)
            STT(srm[:], srm[:], -1.0, yr[:, SL], op0=OP.mult, op1=OP.add)
            STT(sim_[:], sim_[:], -1.0, yi[:, SL], op0=OP.mult, op1=OP.add)
            G.tensor_add(s1[:], s1[:], _bc_k(srm[:], M, KC))  # err_r
            G.tensor_add(s2[:], s2[:], _bc_k(sim_[:], M, KC))  # err_i
            # --- A: magH, XX2, xi ---
            A.square(s3[:], Hr_c)
            A.square(s4[:], Hi_c)
            V.tensor_add(s3[:], s3[:], s4[:])          # magh
            A.square(s4[:], Xr_c)
            A.square(s5[:], Xi_c)
            V.tensor_add(s4[:], s4[:], s5[:])          # xx2
            V.tensor_add(s5[:], s4[:], vX_c)           # XXv
            G.tensor_mul(s6[:], s3[:], vX_c)           # magh*vX
            G.tensor_mul(s5[:], s5[:], vH_c)
            V.tensor_add(s5[:], s6[:], s5[:])          # tmp
            V.tensor_reduce(stm[:], s5[:].transpose([0, 2, 1]),
                            axis=AX.X, op=OP.add)
            V.tensor_add(stm[:], stm[:], n0[:, SL])    # SN
            STT(s5[:], s5[:], -1.0, _bc_k(stm[:], M, KC),
                op0=OP.mult, op1=OP.add)               # xi_y
            V.tensor_add(s6[:], s5[:], vH_c)           # xi_x
            V.reciprocal_approx_fast(s6[:], s6[:])     # r1
            V.tensor_add(s5[:], s5[:], vX_c)           # xi_h
            V.reciprocal_approx_fast(s5[:], s5[:])     # r2
            if chunk0:
                STT(s5[:], s5[:], 1.0, _bc_k(m2e[:, SL], M, KC),
                    op0=OP.mult, op1=OP.mult)          # w = r2*m2
            # --- C-pre (factored): te2 = w*(X o err), vt2 = w*xx2 ---
            V.tensor_mul(s7[:], Xr_c, s1[:])           # A1
            G.tensor_mul(s8[:], Xi_c, s2[:])           # A2
            V.tensor_add(s7[:], s7[:], s8[:])          # A3
            G.tensor_mul(s8[:], Xr_c, s2[:])           # A4
            G.tensor_mul(s9[:], Xi_c, s1[:])           # A5
            G.tensor_sub(s8[:], s8[:], s9[:])          # A6
            if chunk0:
                V.tensor_mul(te2r[:, :, SL], s7[:], s5[:])
                V.tensor_mul(te2i[:, :, SL], s8[:], s5[:])
                V.tensor_mul(vt2[:, :, SL], s4[:], s5[:])
            else:
                STT(te2r[:, :, SL], s7[:], beta_s, s5[:],
                    op0=OP.mult, op1=OP.mult)
                STT(te2i[:, :, SL], s8[:], beta_s, s5[:],
                    op0=OP.mult, op1=OP.mult)
                STT(vt2[:, :, SL], s4[:], beta_s, s5[:],
                    op0=OP.mult, op1=OP.mult)
            # --- B (factored): te = r1*(H o err), vt = magh*r1 ---
            V.tensor_mul(s4[:], Hr_c, s1[:])           # B1
            G.tensor_mul(s7[:], Hi_c, s2[:])           # B2
            V.tensor_add(s4[:], s4[:], s7[:])          # B3
            G.tensor_mul(s7[:], Hr_c, s2[:])           # B4
            G.tensor_mul(s8[:], Hi_c, s1[:])           # B5
            G.tensor_sub(s7[:], s7[:], s8[:])          # B6
            G.tensor_mul(s3[:], s3[:], s6[:])          # vt = magh*r1
            V.tensor_mul(s4[:], s4[:], s6[:])          # ter
            G.tensor_mul(s7[:], s7[:], s6[:])          # tei
            psv = ps.tile(sh, FP, tag="psv", name="psv")
            psr = ps.tile(sh, FP, tag="psr", name="psr")
            psi = ps.tile(sh, FP, tag="psi", name="psi")
            for src, dst in ((s3, psv), (s4, psr), (s7, psi)):
                sf, df = _flat(src[:]), _flat(dst[:])
                for hh in range(2):
                    HS = slice(hh * 512, (hh + 1) * 512)
                    T.matmul(df[:, HS], bones[:], sf[:, HS],
                             start=True, stop=True)
            if not last:
                STT(s3[:], s3[:], -1.0, psv[:], op0=OP.mult, op1=OP.add)
                V.reciprocal_approx_fast(s3[:], s3[:])  # varX
                STT(s4[:], s4[:], -1.0, psr[:], op0=OP.mult, op1=OP.add)
                STT(s7[:], s7[:], -1.0, psi[:], op0=OP.mult, op1=OP.add)
                V.tensor_mul(s4[:], s4[:], s3[:])       # est_r
                G.tensor_mul(s7[:], s7[:], s3[:])       # est_i
                A.activation(s5[:], s4[:], AF.Tanh, scale=cgam)  # mr
                A.activation(s8[:], s7[:], AF.Tanh, scale=cgam)  # mi
                if chunk0:
                    omb = _bc_k(omc[:, SL], M, KC)
                    s2eb = _bc_k(s2em[:, SL], M, KC)
                    V.tensor_mul(s1[:], Xr_c, omb)
                    STT(s2[:], s2eb, 2.0 * S, s5[:], op0=OP.mult, op1=OP.mult)
                    V.tensor_add(Xr_c, s1[:], s2[:])
                    V.tensor_mul(s1[:], Xi_c, omb)
                    STT(s2[:], s2eb, 2.0 * S, s8[:], op0=OP.mult, op1=OP.mult)
                    V.tensor_add(Xi_c, s1[:], s2[:])
                    A.square(s1[:], s5[:])
                    A.square(s2[:], s8[:])
                    G.tensor_add(s1[:], s1[:], s2[:])   # q2
                    V.tensor_mul(s2[:], vX_c, omb)
                    STT(s6[:], s1[:], -1.0, s2eb, op0=OP.mult, op1=OP.mult)
                    V.tensor_add(s2[:], s2[:], s6[:])
                    STT(vX_c, s2eb, 2.0, s2[:], op0=OP.mult, op1=OP.add)
                else:
                    A.activation(s1[:], s5[:], AF.Identity, scale=seta_s)
                    STT(Xr_c, Xr_c, ometa_s, s1[:], op0=OP.mult, op1=OP.add)
                    A.activation(s1[:], s8[:], AF.Identity, scale=seta_s)
                    STT(Xi_c, Xi_c, ometa_s, s1[:], op0=OP.mult, op1=OP.add)
                    A.square(s1[:], s5[:])
                    A.square(s2[:], s8[:])
                    G.tensor_add(s1[:], s1[:], s2[:])   # q2
                    A.activation(s2[:], s1[:], AF.Identity,
                                 scale=ns2e_s, bias=eta_s)
                    STT(vX_c, vX_c, ometa_s, s2[:], op0=OP.mult, op1=OP.add)
            else:
                V.reciprocal_approx_accurate(s3[:], psv[:], s9[:])
                V.tensor_mul(s4[:], psr[:], s3[:])      # est_r
                V.tensor_mul(s7[:], psi[:], s3[:])      # est_i
                A.activation(s5[:], s4[:], AF.Sign)     # sgn_r
                A.activation(s8[:], s7[:], AF.Sign)     # sgn_i
                smb = _bc_k(smaskc[:, SL], M, KC)
                V.tensor_mul(s4[:], s5[:], smb)         # Xp_r
                V.tensor_mul(s7[:], s8[:], smb)         # Xp_i
                nc.sync.dma_start(xp_r_d[:, :, SL], s4[0:P:N, :, :])
                nc.sync.dma_start(xp_i_d[:, :, SL], s7[0:P:N, :, :])
                # vp = (1 - epr^2) - epi^2 (ref op order)
                A.mul(s5[:], s5[:], S)                  # epr
                V.tensor_mul(s5[:], s5[:], s5[:])
                A.activation(s5[:], s5[:], AF.Identity, scale=-1.0, bias=1.0)
                A.mul(s8[:], s8[:], S)
                G.tensor_mul(s8[:], s8[:], s8[:])
                V.tensor_sub(s5[:], s5[:], s8[:])       # vp
                V.tensor_mul(s5[:], s5[:], _bc_k(maskc[:, SL], M, KC))
                nc.sync.dma_start(vxp_d[:, :, SL], s5[0:P:N, :, :])

        # ================= mega2: VN_H =================
        if not last:
            svF = sm.tile([P, M], FP, tag="svF", name="svF")
            stF = sm.tile([P, M], FP, tag="stF", name="stF")
            siF = sm.tile([P, M], FP, tag="siF", name="siF")
            V.tensor_reduce(svF[:], vt2[:], axis=AX.X, op=OP.add)
            V.tensor_reduce(stF[:], te2r[:], axis=AX.X, op=OP.add)
            V.tensor_reduce(siF[:], te2i[:], axis=AX.X, op=OP.add)
            A.add(svF[:], svF[:], 1.0)                  # Sv+1
            for jj in range(M // MC):
                JS = slice(jj * MC, (jj + 1) * MC)
                shj = [P, MC, K]
                v_j = vt2[:, JS, :]
                r_j = te2r[:, JS, :]
                i_j = te2i[:, JS, :]
                sv = svF[:, JS]
                st_ = stF[:, JS]
                si2 = siF[:, JS]
                qg = scr.tile(shj, FP, tag="s1", name="qg")
                qh = scr.tile(shj, FP, tag="s2", name="qh")
                qi = scr.tile(shj, FP, tag="s3", name="qi")
                STT(qg[:], v_j, -1.0, _bc_inner(sv[:], K),
                    op0=OP.mult, op1=OP.add)
                V.reciprocal_approx_fast(qg[:], qg[:])
                A.activation(qg[:], qg[:], AF.Identity, scale=eta_s)  # gve
                STT(qh[:], r_j, -1.0, _bc_inner(st_[:], K),
                    op0=OP.mult, op1=OP.add)
                V.tensor_mul(qh[:], qh[:], qg[:])
                G.scalar_tensor_tensor(qi[:], i_j, -1.0,
                                       _bc_inner(si2[:], K),
                                       op0=OP.mult, op1=OP.add)
                G.tensor_mul(qi[:], qi[:], qg[:])
                G.scalar_tensor_tensor(Hr[:, JS, :], Hr[:, JS, :], ometa_s,
                                       qh[:], op0=OP.mult, op1=OP.add)
                STT(Hi[:, JS, :], Hi[:, JS, :], ometa_s, qi[:],
                    op0=OP.mult, op1=OP.add)
                G.scalar_tensor_tensor(vH[:, JS, :], vH[:, JS, :], ometa_s,
                                       qg[:], op0=OP.mult, op1=OP.add)
        else:
            sv = sm.tile([P, M], FP, tag="svL", name="sv")
            st_ = sm.tile([P, M], FP, tag="stL", name="st_")
            si2 = sm.tile([P, M], FP, tag="siL", name="si2")
            g1 = sm.tile([P, M], FP, tag="g1", name="g1")
            g2 = sm.tile([P, M], FP, tag="g2", name="g2")
            h1 = sm.tile([P, M], FP, tag="h1", name="h1")
            V.tensor_reduce(sv[:], vt2[:], axis=AX.X, op=OP.add)
            V.tensor_reduce(st_[:], te2r[:], axis=AX.X, op=OP.add)
            V.tensor_reduce(si2[:], te2i[:], axis=AX.X, op=OP.add)
            V.reciprocal_approx_accurate(g1[:], sv[:], g2[:])  # varH
            A.add(g2[:], g1[:], 1.0)                    # 1+varH
            V.reciprocal_approx_accurate(h1[:], g2[:], sv[:])  # g
            V.tensor_mul(st_[:], st_[:], g1[:])         # estHr
            V.tensor_mul(si2[:], si2[:], g1[:])         # estHi
            V.tensor_mul(st_[:], st_[:], h1[:])         # Hp_r
            V.tensor_mul(si2[:], si2[:], h1[:])         # Hp_i
            V.tensor_mul(g1[:], g1[:], h1[:])           # vHp
            nc.sync.dma_start(hp_r_d, st_[:])
            nc.sync.dma_start(hp_i_d, si2[:])
            nc.sync.dma_start(vhp_d, g1[:])


def _host_prep(inputs):
    H_est = np.asarray(inputs["H_est"])
    X_est = np.asarray(inputs["X_est"])
    var_X = np.asarray(inputs["var_X"], np.float32)
    var_H = np.asarray(inputs["var_H"], np.float32)
    Y = np.asarray(inputs["Y"])
    N0 = np.asarray(inputs["N0"], np.float32)
    alphas = np.asarray(inputs["alphas"], np.float32)
    betas = np.asarray(inputs["betas"], np.float32)
    gammas = np.asarray(inputs["gammas"], np.float32)
    etas = np.asarray(inputs["etas"], np.float32)

    mask_g = np.ones((B, K), np.float32)
    for b in range(B):
        if (b % C) < CP:
            mask_g[b, :KP] = 0.0

    bones = np.zeros((P, P), np.float32)
    for g in range(BPC):
        bones[g * N:(g + 1) * N, g * N:(g + 1) * N] = 1.0

    Sf = np.float32(S)
    in_maps = []
    for c in range(NCORES):
        bsl = slice(c * BPC, (c + 1) * BPC)
        xr = np.broadcast_to(X_est[bsl].real[:, None], (BPC, N, M, K))
        xi = np.broadcast_to(X_est[bsl].imag[:, None], (BPC, N, M, K))
        vx = np.broadcast_to(var_X[bsl][:, None], (BPC, N, M, K))
        maskp = np.repeat(mask_g[bsl], N, axis=0)          # [128, K]
        cdata = np.stack([
            Y[bsl].real.reshape(P, K).astype(np.float32),
            Y[bsl].imag.reshape(P, K).astype(np.float32),
            N0[bsl].reshape(P, K),
            maskp,
            Sf * maskp,
        ], axis=1)
        lc = np.zeros((P, 10, K), np.float32)
        for lay in range(3):
            em = etas[lay] * maskp
            lc[:, 3 * lay + 0] = alphas[lay] * (1.0 - maskp) + betas[lay] * maskp
            lc[:, 3 * lay + 1] = 1.0 - em
            lc[:, 3 * lay + 2] = (Sf * Sf) * em
        lc[:, 9] = alphas[3] * (1.0 - maskp) + betas[3] * maskp
        scm = np.zeros((P, 24), np.float32)
        for lay in range(3):
            scm[:, lay] = np.float32(2.0) * Sf / gammas[lay]
            scm[:, 3 + lay] = etas[lay]
            scm[:, 6 + lay] = np.float32(1.0) - etas[lay]
            scm[:, 9 + lay] = Sf * etas[lay]
            scm[:, 12 + lay] = -(Sf * Sf) * etas[lay]
            scm[:, 15 + lay] = betas[lay]
        scm[:, 18] = betas[3]
        in_maps.append({
            "xr": np.ascontiguousarray(xr.reshape(P, M, K), np.float32),
            "xi": np.ascontiguousarray(xi.reshape(P, M, K), np.float32),
            "vx": np.ascontiguousarray(vx.reshape(P, M, K), np.float32),
            "hr": np.ascontiguousarray(H_est[bsl].real.reshape(P, M), np.float32),
            "hi": np.ascontiguousarray(H_est[bsl].imag.reshape(P, M), np.float32),
            "vh": np.ascontiguousarray(var_H[bsl].reshape(P, M), np.float32),
            "cdata": np.ascontiguousarray(cdata),
            "lc": np.ascontiguousarray(lc),
            "sc": np.ascontiguousarray(scm),
            "bones": bones,
        })
    return in_maps


def kernel(**inputs):
    nc = _build()
    in_maps = _host_prep(inputs)
    res = run_bass_kernel_spmd(nc, in_maps, list(range(NCORES))).results
    hp = np.empty((B, N, M), np.complex64)
    xp = np.empty((B, M, K), np.complex64)
    vxp = np.empty((B, M, K), np.float32)
    vhp = np.empty((B, N, M), np.float32)
    for c in range(NCORES):
        bsl = slice(c * BPC, (c + 1) * BPC)
        r = res[c]
        hp[bsl] = (r["hp_r"] + 1j * r["hp_i"]).reshape(BPC, N, M)
        vhp[bsl] = r["vhp"].reshape(BPC, N, M)
        xp[bsl] = r["xp_r"] + 1j * r["xp_i"]
        vxp[bsl] = r["vxp"]
    return hp, xp, vxp, vhp
